# revision 9
# baseline (speedup 1.0000x reference)
"""Trainium2 Bass kernel for nn_Attention_26792005992653.

Full-input contract: kernel(**inputs) takes the complete unsharded inputs and
returns the full [2, 2048, 128] output. Internally shards across 8 NeuronCores:
data-parallel over batch (2) x tensor-parallel over heads (16 -> 4 groups of 4).
Each core computes a per-(batch, head-group) partial of the output projection
in transposed layout [128, 2048]; the host sums head-group partials, adds the
output bias, and applies the final cube.

Per-core pipeline (all layouts chosen to avoid on-chip transposes of the big
attention tensors):
  1. x [2048,1024] loaded naturally, transposed 128x128-wise on the PE into
     xT [1024, 2048] (fp32, exact).
  2. QKV projection via PE matmuls in fp32r (full-rate fp32 with ~1e-4 rel
     error). q,k produced *transposed* [d, tok] and cast to bf16 (score
     precision is insensitive: |scores| ~ 0.02, so softmax weights stay
     ~1+s); v produced naturally [tok, x] in fp32, v_bias folded in via a
     K=1 ones-row matmul.
  3. Rotary applied to qT/kT in [d, tok] layout with host-precomputed
     transposed cos/sin tables (bf16) on the vector engine.
  4. Attention per head with *transposed* scores sT[k, q] = kT.T@qT chunks so
     the key mask is a per-partition activation bias (exp(-3e4) == 0 exactly)
     and softmax needs no free-axis broadcast. exp on the scalar engine
     PSUM->SBUF. o^T accumulates over k-tiles via AV matmuls (v as stationary
     operand, p as moving); denominators via an all-ones stationary matmul
     into a broadcast [128, q] PSUM tile. Normalization multiplies o^T by the
     approx-reciprocal of the denominator; query-row masking is applied at the
     very end on the projected output.
  5. Output projection in transposed form outT[y, q] += W_h.T @ (o^T_norm)^3,
     accumulating all 4 local heads in PSUM.
"""

import numpy as np
import ml_dtypes

import concourse.bass as bass
import concourse.bacc as bacc
import concourse.tile as tile
import concourse.mybir as mybir
from concourse.bass_utils import run_bass_kernel_spmd

F32 = mybir.dt.float32
F32R = mybir.dt.float32r
BF16 = mybir.dt.bfloat16

B, S, DI = 2, 2048, 1024
NH, DQK, DX = 16, 128, 128
H = 4                     # heads per core
N_CORES = 8
NT = S // 128             # 16 token tiles
NIC = DI // 128           # 8 contraction chunks of 128
QC = 1024                 # query chunk in attention stage
NQC = S // QC             # 2
INV_SQRT_D = 1.0 / float(np.sqrt(np.float32(DQK)))
MASK_BIAS = -30000.0

AF = mybir.ActivationFunctionType


def r(ap):
    """fp32r tiles are declared as such; kept for call-site clarity"""
    return ap


def _build_body(nc, tc, dram):
    from contextlib import ExitStack

    x_d, wqk_d, wv_d, vb_d, wo_d, cos_d, sin_d, kbias_d, keepb_d, ones_d, out_d = dram

    with ExitStack() as ctx:
        consts = ctx.enter_context(tc.tile_pool(name="consts", bufs=1))
        qkT_pool = ctx.enter_context(tc.tile_pool(name="qkT", bufs=1))
        v_pool = ctx.enter_context(tc.tile_pool(name="v", bufs=1))

        # ---- constants ----
        cosT = consts.tile([128, S], BF16, tag="cosT", name="cosT")
        sinT = consts.tile([128, S], BF16, tag="sinT", name="sinT")
        kbias = consts.tile([128, NT], F32, tag="kbias", name="kbias")
        keepb = consts.tile([128, S], F32, tag="keepb", name="keepb")
        vb = consts.tile([1, H * DX], F32R, tag="vb", name="vb")
        nc.sync.dma_start(out=cosT[:], in_=cos_d[:])
        nc.sync.dma_start(out=sinT[:], in_=sin_d[:])
        nc.sync.dma_start(out=kbias[:], in_=kbias_d[:])
        nc.sync.dma_start(out=keepb[:], in_=keepb_d[:])
        nc.sync.dma_start(out=vb[:], in_=vb_d[:])
        wo = []
        for h in range(H):
            t = consts.tile([128, 128], F32R, tag=f"wo{h}", name=f"wo{h}")
            nc.sync.dma_start(out=t[:], in_=wo_d[h])
            wo.append(t)
        ident = consts.tile([128, 128], F32, tag="ident", name="ident")
        from concourse.masks import make_identity
        make_identity(nc, ident[:])
        ones = consts.tile([128, 128], F32R, tag="ones", name="ones")
        nc.sync.dma_start(out=ones[:], in_=ones_d[:])

        qT = [qkT_pool.tile([128, S], BF16, tag=f"qT{h}", name=f"qT{h}")
              for h in range(H)]
        kT = [qkT_pool.tile([128, S], BF16, tag=f"kT{h}", name=f"kT{h}")
              for h in range(H)]
        vt = [v_pool.tile([128, H * DX], F32R, tag=f"v{t}", name=f"v{t}")
              for t in range(NT)]

        with tc.tile_pool(name="xT", bufs=1) as xT_pool:
            xT = [xT_pool.tile([128, S], F32R, tag=f"xT{c}", name=f"xT{c}")
                  for c in range(NIC)]

            # ---- stage 1: load x (groups of 4 token-tiles), PE-transpose ----
            with tc.tile_pool(name="xn", bufs=8) as xn_pool, \
                 tc.tile_pool(name="ps1", bufs=2, space="PSUM") as ps1:
                for tb in range(NT // 4):
                    xg = []
                    for j in range(4):
                        t = tb * 4 + j
                        xt = xn_pool.tile([128, DI], F32, tag="xn", name=f"xn{t}")
                        nc.sync.dma_start(out=xt[:], in_=x_d[t * 128:(t + 1) * 128, :])
                        xg.append(xt)
                    for c in range(NIC):
                        pt = ps1.tile([128, 512], F32, tag="pt", name="pt")
                        for j in range(4):
                            nc.tensor.transpose(
                                pt[:, j * 128:(j + 1) * 128],
                                xg[j][:, c * 128:(c + 1) * 128],
                                ident[:])
                        nc.scalar.copy(xT[c][:, tb * 512:(tb + 1) * 512], pt[:])

            # ---- stage 2a: QK projection (fp32r) + bf16 cast + rotary ----
            with tc.tile_pool(name="wqk", bufs=2) as wqk_pool, \
                 tc.tile_pool(name="rot", bufs=2) as rot_pool, \
                 tc.tile_pool(name="ps2", bufs=2, space="PSUM") as ps2:
                for h in range(H):
                    wqk = wqk_pool.tile([128, NIC, 2 * DQK], F32R, tag="wqk", name="wqk")
                    nc.sync.dma_start(
                        out=wqk[:],
                        in_=wqk_d[h].rearrange("(c p) d -> p c d", p=128))
                    for qk, dst in ((0, qT[h]), (1, kT[h])):
                        for tc4 in range(4):
                            pq = ps2.tile([128, 512], F32, tag="pq", name="pq")
                            for c in range(NIC):
                                nc.tensor.matmul(
                                    pq[:],
                                    r(wqk[:, c, qk * 128:(qk + 1) * 128]),
                                    r(xT[c][:, tc4 * 512:(tc4 + 1) * 512]),
                                    start=(c == 0), stop=(c == NIC - 1))
                            nc.scalar.copy(dst[:, tc4 * 512:(tc4 + 1) * 512], pq[:])
                        # rotary in [d, tok] layout: rows 0:64 pair with 64:128
                        rt = rot_pool.tile([128, S], BF16, tag="rt", name="rt")
                        nc.vector.tensor_scalar_mul(rt[0:64, :], dst[64:128, :], -1.0)
                        nc.vector.tensor_copy(rt[64:128, :], dst[0:64, :])
                        nc.vector.tensor_mul(dst[:], dst[:], cosT[:])
                        nc.vector.tensor_mul(rt[:], rt[:], sinT[:])
                        nc.vector.tensor_add(dst[:], dst[:], rt[:])

            # ---- stage 2b: V projection (fp32r, natural layout) + bias ----
            with tc.tile_pool(name="wv", bufs=1) as wv_pool, \
                 tc.tile_pool(name="ps3", bufs=2, space="PSUM") as ps3:
                wv = wv_pool.tile([128, NIC, H * DX], F32R, tag="wv", name="wv")
                nc.sync.dma_start(out=wv[:], in_=wv_d.rearrange("(c p) d -> p c d", p=128))
                for t in range(NT):
                    pv = ps3.tile([128, H * DX], F32, tag="pv", name="pv")
                    for c in range(NIC):
                        nc.tensor.matmul(
                            pv[:],
                            r(xT[c][:, t * 128:(t + 1) * 128]),
                            r(wv[:, c, :]),
                            start=(c == 0), stop=False)
                    nc.tensor.matmul(pv[:], r(ones[0:1, :]), r(vb[:]),
                                     start=False, stop=True)
                    nc.scalar.copy(vt[t][:], pv[:])

        # ---- stage 3: attention ----
        with tc.tile_pool(name="p", bufs=3) as p_pool, \
             tc.tile_pool(name="o3", bufs=1) as o3_pool, \
             tc.tile_pool(name="att_tmp", bufs=2) as tmp_pool, \
             tc.tile_pool(name="outsb", bufs=2) as out_pool, \
             tc.tile_pool(name="ps_s", bufs=2, space="PSUM") as pss, \
             tc.tile_pool(name="ps_o", bufs=1, space="PSUM") as pso, \
             tc.tile_pool(name="ps_d", bufs=1, space="PSUM") as psd:
            o3 = {}
            for h in range(H):
                for qc in range(NQC):
                    ps_o = pso.tile([128, QC], F32, tag="ps_o", name="ps_o")
                    ps_d = psd.tile([128, QC], F32, tag="ps_d", name="ps_d")
                    for kt in range(NT):
                        ps_s = pss.tile([128, QC], F32, tag="ps_s", name="ps_s")
                        for j in range(QC // 512):
                            nc.tensor.matmul(
                                ps_s[:, j * 512:(j + 1) * 512],
                                kT[h][:, kt * 128:(kt + 1) * 128],
                                qT[h][:, qc * QC + j * 512: qc * QC + (j + 1) * 512],
                                start=True, stop=True)
                        p = p_pool.tile([128, QC], F32R, tag="p", name="p")
                        nc.scalar.activation(p[:], ps_s[:], AF.Exp,
                                             bias=kbias[:, kt:kt + 1],
                                             scale=INV_SQRT_D)
                        for j in range(QC // 512):
                            sl = slice(j * 512, (j + 1) * 512)
                            nc.tensor.matmul(
                                ps_o[:, sl],
                                r(vt[kt][:, h * DX:(h + 1) * DX]),
                                r(p[:, sl]),
                                start=(kt == 0), stop=(kt == NT - 1))
                            nc.tensor.matmul(
                                ps_d[:, sl],
                                r(ones[:]),
                                r(p[:, sl]),
                                start=(kt == 0), stop=(kt == NT - 1))
                    rec = tmp_pool.tile([128, QC], F32, tag="rec", name="rec")
                    nc.vector.reciprocal_approx_fast(rec[:], ps_d[:])
                    on = tmp_pool.tile([128, QC], F32, tag="on", name="on")
                    nc.vector.tensor_mul(on[:], ps_o[:], rec[:])
                    sq = tmp_pool.tile([128, QC], F32, tag="sq", name="sq")
                    nc.scalar.square(sq[:], on[:])
                    o3t = o3_pool.tile([128, QC], F32R, tag=f"o3_{h}_{qc}",
                                       name=f"o3_{h}_{qc}")
                    nc.vector.tensor_mul(o3t[:], sq[:], on[:])
                    o3[(h, qc)] = o3t

            # ---- output projection, transposed: outT[y, q] ----
            for qc in range(NQC):
                ps_out = pss.tile([128, QC], F32, tag="ps_s", name="ps_out")
                for h in range(H):
                    for j in range(QC // 512):
                        sl = slice(j * 512, (j + 1) * 512)
                        nc.tensor.matmul(
                            ps_out[:, sl],
                            r(wo[h][:]),
                            r(o3[(h, qc)][:, sl]),
                            start=(h == 0), stop=(h == H - 1))
                outsb = out_pool.tile([128, QC], F32, tag="outsb", name="outsb")
                nc.vector.tensor_mul(outsb[:], ps_out[:],
                                     keepb[:, qc * QC:(qc + 1) * QC])
                nc.sync.dma_start(out=out_d[:, qc * QC:(qc + 1) * QC], in_=outsb[:])


def build_nc():
    nc = bacc.Bacc("TRN2", target_bir_lowering=False, debug=False)
    x_d = nc.declare_dram_parameter("x", [S, DI], F32, isOutput=False)
    wqk_d = nc.declare_dram_parameter("wqk", [H, DI, 2 * DQK], F32R, isOutput=False)
    wv_d = nc.declare_dram_parameter("wv", [DI, H * DX], F32R, isOutput=False)
    vb_d = nc.declare_dram_parameter("vb", [1, H * DX], F32R, isOutput=False)
    wo_d = nc.declare_dram_parameter("wo", [H, DX, DX], F32R, isOutput=False)
    cos_d = nc.declare_dram_parameter("cosT", [128, S], BF16, isOutput=False)
    sin_d = nc.declare_dram_parameter("sinT", [128, S], BF16, isOutput=False)
    kbias_d = nc.declare_dram_parameter("kbias", [128, NT], F32, isOutput=False)
    keepb_d = nc.declare_dram_parameter("keepb", [128, S], F32, isOutput=False)
    ones_d = nc.declare_dram_parameter("ones", [128, 128], F32R, isOutput=False)
    out_d = nc.declare_dram_parameter("outT", [128, S], F32, isOutput=True)
    dram = (x_d, wqk_d, wv_d, vb_d, wo_d, cos_d, sin_d, kbias_d, keepb_d, ones_d, out_d)
    with tile.TileContext(nc) as tc:
        _build_body(nc, tc, dram)
    nc.compile()
    return nc


_NC = None


def _get_nc():
    global _NC
    if _NC is None:
        _NC = build_nc()
    return _NC


def _rotary_tables():
    half = DQK // 2
    freq_half = (10000.0 ** (np.arange(half, dtype=np.float32) * np.float32(-2.0 / DQK))).astype(np.float32)
    freq = np.concatenate([freq_half, freq_half])          # [128]
    pos = np.arange(S, dtype=np.float32)
    ang = pos[None, :] * freq[:, None]                     # [128, S] (transposed)
    return (np.cos(ang).astype(ml_dtypes.bfloat16),
            np.sin(ang).astype(ml_dtypes.bfloat16))


def make_in_maps(x, mask, proj_in, v_bias, proj_out):
    cosT, sinT = _rotary_tables()
    x = np.asarray(x, dtype=np.float32)
    mask = np.asarray(mask)
    proj_in = np.asarray(proj_in, dtype=np.float32)
    v_bias = np.asarray(v_bias, dtype=np.float32)
    proj_out = np.asarray(proj_out, dtype=np.float32)

    in_maps = []
    for core in range(N_CORES):
        b, hg = divmod(core, N_CORES // B)
        heads = slice(hg * H, (hg + 1) * H)
        wqk = np.ascontiguousarray(
            proj_in[:, heads, :2 * DQK].transpose(1, 0, 2))          # [H, DI, 256]
        wv = np.ascontiguousarray(
            proj_in[:, heads, 2 * DQK:].reshape(DI, H * DX))         # [DI, (h,x)]
        vb = np.ascontiguousarray(v_bias[heads].reshape(1, H * DX))
        wo = np.ascontiguousarray(proj_out[heads])                   # [H, 128, 128]
        mb = mask[b]                                                 # [S] bool
        kbias = np.where(mb.reshape(NT, 128).T, MASK_BIAS, 0.0).astype(np.float32)
        keepb = np.broadcast_to(
            np.where(mb, 0.0, 1.0).astype(np.float32)[None, :], (128, S)).copy()
        in_maps.append({
            "x": np.ascontiguousarray(x[b]),
            "wqk": wqk, "wv": wv, "vb": vb, "wo": wo,
            "cosT": cosT, "sinT": sinT,
            "kbias": kbias, "keepb": keepb,
            "ones": np.ones((128, 128), dtype=np.float32),
        })
    return in_maps


def gather(results, proj_out_bias):
    out = np.empty((B, S, DX), dtype=np.float32)
    g = N_CORES // B
    for b in range(B):
        acc = results[b * g]["outT"].T.astype(np.float32).copy()
        for hg in range(1, g):
            acc += results[b * g + hg]["outT"].T
        acc += np.asarray(proj_out_bias, dtype=np.float32)[None, :]
        out[b] = acc ** 3
    return out


def run(inputs, trace=False, trace_cores=None):
    nc = _get_nc()
    in_maps = make_in_maps(inputs["x"], inputs["mask"], inputs["proj_in"],
                           inputs["v_bias"], inputs["proj_out"])
    res = run_bass_kernel_spmd(nc, in_maps, list(range(N_CORES)),
                               trace=trace, trace_cores=trace_cores)
    out = gather(res.results, inputs["proj_out_bias"])
    return out, res


def kernel(x, mask, proj_in, v_bias, proj_out, proj_out_bias):
    out, _ = run({"x": x, "mask": mask, "proj_in": proj_in, "v_bias": v_bias,
                  "proj_out": proj_out, "proj_out_bias": proj_out_bias})
    return out
